# revision 17
# baseline (speedup 1.0000x reference)
"""Trainium2 Bass kernel for nn_Attention_19877108646354 (aspect-attention pooling).

Math (per batch b):
    th = hidden[b] @ Wh_w.T + Wh_b            # [S, H]
    u  = tanh(th) @ w_w[0, :H]                # [S]   (aspect branch + w_b are
                                              #        constant per batch -> cancel in softmax)
    alpha = softmax(u)                        # [S]
    r[b]  = alpha @ hidden[b]                 # [H]

Sharding: data-parallel over batch, 4 batches per core on 8 cores.

On-device pipeline per batch:
  1. SWDGE cast-DMA: hidden[b] fp32 DRAM -> natural bf16 SBUF  [128s, 8x1024h] x2 halves
  2. PE: transpose via normal matmul against identity (stays HAM-warm):
       hiddenT[h-tile][128h, s] bf16, evicted PSUM->SBUF by DVE cast-copies
  3. PE mm1: th.T[g,s] = sum_h WhT[h,g-tile].T @ hiddenT -> PSUM [128g, 512s]
  4. ACT: tanh(th.T + Wh_b[g]) PSUM -> SBUF bf16
  5. PE u-mm: u[1, 512s] += w[g-tile].T @ tanh  (accumulate over g in PSUM)
  6. ACT: e = exp(u) (no max-shift needed, |u| <= ~1.5) with accum_out = sum(e)
  7. DVE: rz = 1/sum(e);  SWDGE strided DMA reshapes e [1,2048] -> eT [128,16]
  8. 2nd pass: HWDGE fp32 loads of hidden[b]; PE mm2 (float32r):
       r_unnorm[1, 1024] += eT[:, st].T @ hidden_tile
  9. ACT: r = r_unnorm * rz -> SBUF; DMA to output.
"""

from contextlib import ExitStack

import numpy as np
import ml_dtypes

import concourse.bass as bass
import concourse.tile as tile
import concourse.mybir as mybir
from concourse.bass_utils import run_bass_kernel_spmd

B, S, H, A = 32, 2048, 1024, 256
NCORES = 8
BPC = B // NCORES          # batches per core
ST = S // 128              # 16 s-tiles per batch
HT = H // 128              # 8 h-tiles
GT = H // 128              # 8 g-tiles
SC = S // 512              # 4 s-chunks of 512

F32 = mybir.dt.float32
F32R = mybir.dt.float32r
BF16 = mybir.dt.bfloat16
AF = mybir.ActivationFunctionType

_nop_uid = [0]


class SplitWaitTC(tile.TileContext):
    """TileContext variant for a walrus codegen that accepts at most ONE sync
    wait per instruction: extra waits are peeled onto same-engine NoOps placed
    immediately before the instruction (semantically identical), and the tail
    drain's many-lane wait set is spread over SP NoOps."""

    def _add_instruction(self, inst):
        si = inst.sync_info
        if si is not None and len(si.on_wait) > 1:
            waits = list(si.on_wait)
            for w in waits[:-1]:
                _nop_uid[0] += 1
                nop = mybir.InstNoOp(
                    name=f"waitsplit_{_nop_uid[0]}",
                    sync_info=mybir.SyncInfo(on_wait=[w], on_update=[]),
                    bass_nofuse=True,
                    engine=inst.engine,
                )
                super()._add_instruction(nop)
            inst.sync_info = mybir.SyncInfo(
                on_wait=[waits[-1]], on_update=list(si.on_update)
            )
        super()._add_instruction(inst)

    def _drain_and_barrier(self, tick_clock, wait_clock):
        from concourse.vector_clock import ScopedClock

        drain_inst = self.nc.sync.drain()
        wait_clock.add_sem_waits(
            drain_inst.ins, ScopedClock({None: tick_clock.global_clock})
        )
        si = drain_inst.ins.sync_info
        if si is not None and len(si.on_wait) > 1:
            waits = list(si.on_wait)
            drain_inst.ins.sync_info = mybir.SyncInfo(
                on_wait=[waits[0]], on_update=list(si.on_update)
            )
            for w in waits[1:]:
                nop = self.nc.sync.nop(nofuse=True, hint="drain_split")
                nop.ins.sync_info = mybir.SyncInfo(on_wait=[w], on_update=[])

        self.nc.all_engine_barrier()
        assert self.sems is not None
        popped = self.nc._tile_sem_poison_stack.pop()
        assert popped is self._sem_poison
        self.nc.clear_and_free_semaphores(list(self.sems.allocated().values()))
        self.nc.all_engine_barrier()


def build_kernel(reps=1):
    nc = bass.Bass(trn_type="TRN2")

    hid = nc.dram_tensor("hidden", [BPC, S, H], F32, kind="ExternalInput")
    whT = nc.dram_tensor("whT", [H, H], BF16, kind="ExternalInput")       # WhT[h, g] = Wh_w[g, h]
    whb = nc.dram_tensor("whb", [GT, 128], F32, kind="ExternalInput")     # whb[gt, p] = Wh_b[gt*128+p]
    wcol = nc.dram_tensor("wcol", [GT, 128], BF16, kind="ExternalInput")  # wcol[gt, p] = w_w[0, gt*128+p]
    ident = nc.dram_tensor("ident", [128, 128], BF16, kind="ExternalInput")
    out = nc.dram_tensor("out", [BPC, 1, H], F32, kind="ExternalOutput")

    with SplitWaitTC(nc) as tc, ExitStack() as ctx:
        consts = ctx.enter_context(tc.tile_pool(name="consts", bufs=1))
        nat_pool = ctx.enter_context(tc.tile_pool(name="nat", bufs=3))
        ht_pool = ctx.enter_context(tc.tile_pool(name="hT", bufs=1))
        tanh_pool = ctx.enter_context(tc.tile_pool(name="tanh", bufs=3))
        nat2_pool = ctx.enter_context(tc.tile_pool(name="nat2", bufs=3))
        small_pool = ctx.enter_context(tc.tile_pool(name="small", bufs=2))
        dram_pool = ctx.enter_context(tc.tile_pool(name="dram", bufs=2, space="DRAM"))
        psum_tr = ctx.enter_context(tc.tile_pool(name="ptr", bufs=2, space="PSUM"))
        psum_th = ctx.enter_context(tc.tile_pool(name="pth", bufs=2, space="PSUM"))
        psum_u = ctx.enter_context(tc.tile_pool(name="pu", bufs=2, space="PSUM"))
        psum_r = ctx.enter_context(tc.tile_pool(name="pr", bufs=1, space="PSUM"))

        # --- load constants ---
        whT_sb = consts.tile([128, HT, H], BF16)      # [p(h), ht, g]
        for ht in range(HT):
            nc.sync.dma_start(whT_sb[:, ht, :], whT[ht * 128:(ht + 1) * 128, :])
        whb_sb = consts.tile([128, GT], F32)          # [p(g), gt]
        nc.sync.dma_start(whb_sb[:, :], whb.rearrange("g p -> p g"))
        wcol_sb = consts.tile([128, GT], BF16)
        nc.sync.dma_start(wcol_sb[:, :], wcol.rearrange("g p -> p g"))
        ident_sb = consts.tile([128, 128], BF16)
        nc.sync.dma_start(ident_sb[:, :], ident[:, :])

        # hid[b] viewed as [p(s within tile), s-tile, h]
        hid_t = hid.rearrange("b (u p) h -> b p u h", p=128)
        # hid[b] viewed as [p, chunk(256-rows), i, h] for pass 2
        hid_c = hid.rearrange("b (c i p) h -> b p c i h", i=2, p=128)

        for b_iter in range(BPC * reps):
            b = b_iter % BPC
            # ---- pass 1a: load natural bf16 (cast during DMA), four quarter-batches ----
            QS = ST // 4
            nats = []
            for q in range(4):
                nat = nat_pool.tile([128, QS, H], BF16, tag="nat")
                nc.gpsimd.dma_start(
                    nat[:, :, :], hid_t[b, :, q * QS:(q + 1) * QS, :]
                )
                nats.append(nat)

            # ---- pass 1b: transpose to hiddenT bf16 [128h, ht, s] ----
            # loop sq outer so transposes consume quarter-batches as they land
            hT = ht_pool.tile([128, HT, S], BF16, tag="hT")
            for sq in range(4):              # groups of 4 s-tiles = one quarter
                for ht in range(HT):
                    ptr = psum_tr.tile([128, 512], F32, tag="ptr")
                    for k in range(4):
                        st = sq * 4 + k
                        nc.tensor.matmul(
                            ptr[:, k * 128:(k + 1) * 128],
                            lhsT=nats[sq][:, k, ht * 128:(ht + 1) * 128],
                            rhs=ident_sb[:, :],
                            start=True, stop=True,
                        )
                    nc.vector.tensor_copy(
                        hT[:, ht, sq * 512:(sq + 1) * 512], ptr[:, :]
                    )

            # ---- pass 1c: mm1 + tanh + u accumulation ----
            e_sb = small_pool.tile([1, S], F32, tag="e")
            esum4 = small_pool.tile([1, SC], F32, tag="esum4")
            # software-pipelined: u-mm for (sc,g) issues during mm1 of the next
            # group, so it never stalls PE on the just-issued tanh. exp(sc)
            # issues right after sc's final u-mm.
            pending = None   # (pu, sc, g, tanh_sb)

            def flush_pending():
                ppu, psc, pg, ptanh = pending
                nc.tensor.matmul(
                    ppu[0:1, :], lhsT=wcol_sb[:, pg:pg + 1], rhs=ptanh[:, :],
                    start=(pg == 0), stop=(pg == GT - 1),
                )
                if pg == GT - 1:
                    nc.scalar.activation(
                        e_sb[0:1, psc * 512:(psc + 1) * 512], ppu[0:1, :], AF.Exp,
                        accum_out=esum4[0:1, psc:psc + 1],
                    )

            for sc in range(SC):
                pu = psum_u.tile([1, 512], F32, tag="pu")
                for g in range(GT):
                    pth = psum_th.tile([128, 512], F32, tag="pth")
                    for h in range(HT):
                        nc.tensor.matmul(
                            pth[:, :],
                            lhsT=whT_sb[:, h, g * 128:(g + 1) * 128],
                            rhs=hT[:, h, sc * 512:(sc + 1) * 512],
                            start=(h == 0), stop=(h == HT - 1),
                        )
                    if pending is not None:
                        flush_pending()
                    tanh_sb = tanh_pool.tile([128, 512], BF16, tag="tanh")
                    nc.scalar.activation(
                        tanh_sb[:, :], pth[:, :], AF.Tanh,
                        bias=whb_sb[:, g:g + 1],
                    )
                    pending = (pu, sc, g, tanh_sb)
            flush_pending()
            pending = None

            # ---- softmax denominator ----
            esum = small_pool.tile([1, 1], F32, tag="esum")
            nc.vector.tensor_reduce(
                esum[0:1, :], esum4[0:1, :], axis=mybir.AxisListType.X,
                op=mybir.AluOpType.add,
            )
            rz = small_pool.tile([1, 1], F32, tag="rz")
            nc.vector.reciprocal(rz[0:1, :], esum[0:1, :])

            # ---- reshape e [1, 2048] -> eT [128, 16] via DRAM bounce ----
            e_dram = dram_pool.tile([1, S], F32, tag="edram")
            nc.sync.dma_start(e_dram[0:1, :], e_sb[0:1, :])
            eT = small_pool.tile([128, ST], BF16, tag="eT")
            nc.gpsimd.dma_start(
                eT[:, :], e_dram[0:1, :].rearrange("a (t p) -> (a p) t", p=128)
            )

            # ---- pass 2: r_unnorm = e @ hidden (fp32 reload, float32r matmuls) ----
            pr = psum_r.tile([1, H], F32, tag="pr")
            for c in range(8):
                nat2 = nat2_pool.tile([128, 2, H], BF16, tag="nat2")
                nc.gpsimd.dma_start(nat2[:, :, :], hid_c[b, :, c, :, :])
                for i in range(2):
                    st = c * 2 + i
                    for n in range(2):
                        nc.tensor.matmul(
                            pr[0:1, n * 512:(n + 1) * 512],
                            lhsT=eT[:, st:st + 1],
                            rhs=nat2[:, i, n * 512:(n + 1) * 512],
                            start=(st == 0), stop=(st == ST - 1),
                        )
            r_sb = small_pool.tile([1, H], F32, tag="r")
            nc.scalar.activation(r_sb[0:1, :], pr[0:1, :], AF.Copy, scale=rz[0:1, :])
            nc.sync.dma_start(out[b, 0:1, :], r_sb[0:1, :])

    return nc


_NC_CACHE = None


def make_sharded_runner(nc):
    """Build a cached sharded-jit callable for `nc` (mirrors
    bass2jax.run_bass_via_pjrt) so repeated executions can be timed without
    re-jitting. Returns (fn, prep) where prep(in_maps) -> device args and
    fn(*args) -> out arrays."""
    import jax
    import numpy as _np
    from jax.sharding import Mesh, PartitionSpec
    from jax.experimental.shard_map import shard_map
    from concourse import bass2jax as b2j
    import concourse.mybir as _mybir

    b2j.install_neuronx_cc_hook()
    partition_name = nc.partition_id_tensor.name if nc.partition_id_tensor else None
    in_names, out_names, out_avals, zero_outs = [], [], [], []
    for alloc in nc.m.functions[0].allocations:
        if not isinstance(alloc, _mybir.MemoryLocationSet):
            continue
        name = alloc.memorylocations[0].name
        if alloc.kind == "ExternalInput":
            if name != partition_name:
                in_names.append(name)
        elif alloc.kind == "ExternalOutput":
            out_names.append(name)
            shape = tuple(alloc.tensor_shape)
            dtype = _mybir.dt.np(alloc.dtype)
            out_avals.append(jax.core.ShapedArray(shape, dtype))
            zero_outs.append(_np.zeros(shape, dtype))
    n_params = len(in_names)
    n_outs = len(out_avals)
    all_names = in_names + out_names
    if partition_name is not None:
        all_names.append(partition_name)
    donate = tuple(range(n_params, n_params + n_outs))

    def _body(*args):
        operands = list(args)
        if partition_name is not None:
            operands.append(b2j.partition_id_tensor())
        outs = b2j._bass_exec_p.bind(
            *operands,
            out_avals=tuple(out_avals),
            in_names=tuple(all_names),
            out_names=tuple(out_names),
            lowering_input_output_aliases=(),
            sim_require_finite=True,
            sim_require_nnan=True,
            nc=nc,
        )
        return tuple(outs)

    devices = jax.devices()[:NCORES]
    mesh = Mesh(np.asarray(devices), ("core",))
    in_specs = (PartitionSpec("core"),) * (n_params + n_outs)
    out_specs = (PartitionSpec("core"),) * n_outs
    fn = jax.jit(
        shard_map(_body, mesh=mesh, in_specs=in_specs, out_specs=out_specs,
                  check_rep=False),
        donate_argnums=donate, keep_unused=True,
    )

    def prep(in_maps):
        per_core = [[_np.asarray(m[name]) for name in in_names] for m in in_maps]
        concat_in = [
            _np.concatenate([per_core[c][i] for c in range(NCORES)], axis=0)
            for i in range(n_params)
        ]
        dev_in = [jax.device_put(x) for x in concat_in]
        return dev_in

    def zeros():
        return [np.zeros((NCORES * z.shape[0], *z.shape[1:]), z.dtype)
                for z in zero_outs]

    return fn, prep, zeros


def make_chained_runner(nc, k):
    """Like make_sharded_runner but executes the NEFF k times sequentially
    inside ONE jitted program — one tunnel dispatch, k on-device executions.
    Timing two k values isolates pure device time."""
    import jax
    import jax.numpy as jnp
    import numpy as _np
    from jax.sharding import Mesh, PartitionSpec
    from jax.experimental.shard_map import shard_map
    from concourse import bass2jax as b2j
    import concourse.mybir as _mybir

    b2j.install_neuronx_cc_hook()
    partition_name = nc.partition_id_tensor.name if nc.partition_id_tensor else None
    in_names, out_names, out_avals = [], [], []
    for alloc in nc.m.functions[0].allocations:
        if not isinstance(alloc, _mybir.MemoryLocationSet):
            continue
        name = alloc.memorylocations[0].name
        if alloc.kind == "ExternalInput":
            if name != partition_name:
                in_names.append(name)
        elif alloc.kind == "ExternalOutput":
            out_names.append(name)
            out_avals.append(jax.core.ShapedArray(
                tuple(alloc.tensor_shape), _mybir.dt.np(alloc.dtype)))
    n_params = len(in_names)
    all_names = in_names + out_names
    if partition_name is not None:
        all_names.append(partition_name)

    def _body(*args):
        ins = list(args[:n_params])
        outs = list(args[n_params:])
        for _ in range(k):
            operands = ins + outs          # prior outputs seed the out buffers
            if partition_name is not None:
                operands.append(b2j.partition_id_tensor())
            outs = list(b2j._bass_exec_p.bind(
                *operands,
                out_avals=tuple(out_avals),
                in_names=tuple(all_names),
                out_names=tuple(out_names),
                lowering_input_output_aliases=(),
                sim_require_finite=True,
                sim_require_nnan=True,
                nc=nc,
            ))
        return tuple(outs)

    devices = jax.devices()[:NCORES]
    mesh = Mesh(np.asarray(devices), ("core",))
    n_outs = len(out_names)
    in_specs = (PartitionSpec("core"),) * (n_params + n_outs)
    out_specs = (PartitionSpec("core"),) * n_outs
    fn = jax.jit(shard_map(_body, mesh=mesh, in_specs=in_specs,
                           out_specs=out_specs, check_rep=False))

    def prep(in_maps):
        per_core = [[_np.asarray(m[name]) for name in in_names] for m in in_maps]
        concat_in = [
            _np.concatenate([per_core[c][i] for c in range(NCORES)], axis=0)
            for i in range(n_params)
        ]
        concat_in += [
            _np.zeros((NCORES * av.shape[0], *av.shape[1:]), av.dtype)
            for av in out_avals
        ]
        return [jax.device_put(x) for x in concat_in]

    return fn, prep


def kernel(**inputs):
    global _NC_CACHE
    hidden = np.ascontiguousarray(np.asarray(inputs["hidden"], dtype=np.float32))
    Wh_w = np.asarray(inputs["Wh_w"], dtype=np.float32)
    Wh_b = np.asarray(inputs["Wh_b"], dtype=np.float32)
    w_w = np.asarray(inputs["w_w"], dtype=np.float32)

    whT_np = np.ascontiguousarray(Wh_w.T).astype(ml_dtypes.bfloat16)
    whb_np = np.ascontiguousarray(Wh_b.reshape(GT, 128))
    wcol_np = np.ascontiguousarray(w_w[0, :H].reshape(GT, 128)).astype(ml_dtypes.bfloat16)
    ident_np = np.eye(128, dtype=np.float32).astype(ml_dtypes.bfloat16)

    if _NC_CACHE is None:
        _NC_CACHE = build_kernel()
    nc = _NC_CACHE

    in_maps = []
    for k in range(NCORES):
        in_maps.append({
            "hidden": np.ascontiguousarray(hidden[k * BPC:(k + 1) * BPC]),
            "whT": whT_np,
            "whb": whb_np,
            "wcol": wcol_np,
            "ident": ident_np,
        })

    res = run_bass_kernel_spmd(nc, in_maps, core_ids=list(range(NCORES)))
    out = np.concatenate([r["out"] for r in res.results], axis=0)
    return out.astype(np.float32)


if __name__ == "__main__":
    rng = np.random.default_rng(0)
    test_inputs = {
        "hidden": rng.standard_normal((B, S, H), dtype=np.float32),
        "aspect": rng.standard_normal((B, 1, A), dtype=np.float32),
        "Wh_w": rng.standard_normal((H, H), dtype=np.float32) * 0.03,
        "Wh_b": rng.standard_normal((H,), dtype=np.float32) * 0.03,
        "Wv_w": rng.standard_normal((A, A), dtype=np.float32) * 0.06,
        "Wv_b": rng.standard_normal((A,), dtype=np.float32) * 0.06,
        "w_w": rng.standard_normal((1, H + A), dtype=np.float32) * 0.03,
        "w_b": rng.standard_normal((1,), dtype=np.float32) * 0.03,
    }
    r = kernel(**test_inputs)
    print("kernel out", r.shape, r.dtype, float(np.abs(r).max()))


# revision 29
# speedup vs baseline: 1.2411x; 1.2411x over previous
"""Trainium2 Bass kernel for nn_Attention_19877108646354 (aspect-attention pooling).

Math (per batch b):
    th = hidden[b] @ Wh_w.T + Wh_b            # [S, H]
    u  = tanh(th) @ w_w[0, :H]                # [S]   (aspect branch + w_b are
                                              #        constant per batch -> cancel in softmax)
    alpha = softmax(u)                        # [S]
    r[b]  = alpha @ hidden[b]                 # [H]

Sharding: data-parallel over batch, 4 batches per core on 8 cores.

On-device pipeline per batch:
  1. SWDGE cast-DMA: hidden[b] fp32 DRAM -> natural bf16 SBUF  [128s, 8x1024h] x2 halves
  2. PE: transpose via normal matmul against identity (stays HAM-warm):
       hiddenT[h-tile][128h, s] bf16, evicted PSUM->SBUF by DVE cast-copies
  3. PE mm1: th.T[g,s] = sum_h WhT[h,g-tile].T @ hiddenT -> PSUM [128g, 512s]
  4. ACT: tanh(th.T + Wh_b[g]) PSUM -> SBUF bf16
  5. PE u-mm: u[1, 512s] += w[g-tile].T @ tanh  (accumulate over g in PSUM)
  6. ACT: e = exp(u) (no max-shift needed, |u| <= ~1.5) with accum_out = sum(e)
  7. DVE: rz = 1/sum(e);  SWDGE strided DMA reshapes e [1,2048] -> eT [128,16]
  8. 2nd pass: HWDGE fp32 loads of hidden[b]; PE mm2 (float32r):
       r_unnorm[1, 1024] += eT[:, st].T @ hidden_tile
  9. ACT: r = r_unnorm * rz -> SBUF; DMA to output.
"""

from contextlib import ExitStack

import numpy as np
import ml_dtypes

import concourse.bass as bass
import concourse.tile as tile
import concourse.mybir as mybir
from concourse.bass_utils import run_bass_kernel_spmd

B, S, H, A = 32, 2048, 1024, 256
NCORES = 8
BPC = B // NCORES          # batches per core
ST = S // 128              # 16 s-tiles per batch
HT = H // 128              # 8 h-tiles
GT = H // 128              # 8 g-tiles
SC = S // 512              # 4 s-chunks of 512

F32 = mybir.dt.float32
F32R = mybir.dt.float32r
BF16 = mybir.dt.bfloat16
AF = mybir.ActivationFunctionType

_nop_uid = [0]


class SplitWaitTC(tile.TileContext):
    """TileContext variant for a walrus codegen that accepts at most ONE sync
    wait per instruction: extra waits are peeled onto same-engine NoOps placed
    immediately before the instruction (semantically identical), and the tail
    drain's many-lane wait set is spread over SP NoOps."""

    def _add_instruction(self, inst):
        si = inst.sync_info
        if si is not None and len(si.on_wait) > 1:
            waits = list(si.on_wait)
            for w in waits[:-1]:
                _nop_uid[0] += 1
                nop = mybir.InstNoOp(
                    name=f"waitsplit_{_nop_uid[0]}",
                    sync_info=mybir.SyncInfo(on_wait=[w], on_update=[]),
                    bass_nofuse=True,
                    engine=inst.engine,
                )
                super()._add_instruction(nop)
            inst.sync_info = mybir.SyncInfo(
                on_wait=[waits[-1]], on_update=list(si.on_update)
            )
        super()._add_instruction(inst)

    def _drain_and_barrier(self, tick_clock, wait_clock):
        from concourse.vector_clock import ScopedClock

        drain_inst = self.nc.sync.drain()
        wait_clock.add_sem_waits(
            drain_inst.ins, ScopedClock({None: tick_clock.global_clock})
        )
        si = drain_inst.ins.sync_info
        if si is not None and len(si.on_wait) > 1:
            waits = list(si.on_wait)
            drain_inst.ins.sync_info = mybir.SyncInfo(
                on_wait=[waits[0]], on_update=list(si.on_update)
            )
            for w in waits[1:]:
                nop = self.nc.sync.nop(nofuse=True, hint="drain_split")
                nop.ins.sync_info = mybir.SyncInfo(on_wait=[w], on_update=[])

        self.nc.all_engine_barrier()
        assert self.sems is not None
        popped = self.nc._tile_sem_poison_stack.pop()
        assert popped is self._sem_poison
        self.nc.clear_and_free_semaphores(list(self.sems.allocated().values()))
        self.nc.all_engine_barrier()


def build_kernel(reps=1, skip_transpose=False, skip_pass2=False, skip_umm=False,
                 half_dma=False):
    ablation = skip_transpose or skip_pass2 or skip_umm or half_dma
    nc = bass.Bass(trn_type="TRN2")

    hid = nc.dram_tensor("hidden", [BPC, S, H], F32, kind="ExternalInput")
    whT = nc.dram_tensor("whT", [H, H], BF16, kind="ExternalInput")       # WhT[h, g] = Wh_w[g, h]
    whb = nc.dram_tensor("whb", [GT, 128], F32, kind="ExternalInput")     # whb[gt, p] = Wh_b[gt*128+p]
    wcol = nc.dram_tensor("wcol", [GT, 128], BF16, kind="ExternalInput")  # wcol[gt, p] = w_w[0, gt*128+p]
    ident = nc.dram_tensor("ident", [128, 128], BF16, kind="ExternalInput")
    out = nc.dram_tensor("out", [BPC, 1, H], F32, kind="ExternalOutput")

    with SplitWaitTC(nc) as tc, ExitStack() as ctx:
        if ablation:
            tc.race_detector_enabled = False
        consts = ctx.enter_context(tc.tile_pool(name="consts", bufs=1))
        nat_pool = ctx.enter_context(tc.tile_pool(name="nat", bufs=9))
        ht_pool = ctx.enter_context(tc.tile_pool(name="hT", bufs=1))
        tanh_pool = ctx.enter_context(tc.tile_pool(name="tanh", bufs=18))
        small_pool = ctx.enter_context(tc.tile_pool(name="small", bufs=2))
        dram_pool = ctx.enter_context(tc.tile_pool(name="dram", bufs=2, space="DRAM"))
        psum_tr = ctx.enter_context(tc.tile_pool(name="ptr", bufs=2, space="PSUM"))
        psum_th = ctx.enter_context(tc.tile_pool(name="pth", bufs=2, space="PSUM"))
        psum_u = ctx.enter_context(tc.tile_pool(name="pu", bufs=2, space="PSUM"))
        psum_r = ctx.enter_context(tc.tile_pool(name="pr", bufs=1, space="PSUM"))

        # --- load constants ---
        whT_sb = consts.tile([128, HT, H], BF16)      # [p(h), ht, g]
        for ht in range(HT):
            nc.sync.dma_start(whT_sb[:, ht, :], whT[ht * 128:(ht + 1) * 128, :])
        whb_sb = consts.tile([128, GT], F32)          # [p(g), gt]
        nc.sync.dma_start(whb_sb[:, :], whb.rearrange("g p -> p g"))
        wcol_sb = consts.tile([128, GT], BF16)
        nc.sync.dma_start(wcol_sb[:, :], wcol.rearrange("g p -> p g"))
        ident_sb = consts.tile([128, 128], BF16)
        nc.sync.dma_start(ident_sb[:, :], ident[:, :])

        # hid[b] viewed as [p(s within tile), s-tile, h]
        hid_t = hid.rearrange("b (u p) h -> b p u h", p=128)

        for b_iter in range(BPC * reps):
            b = b_iter % BPC
            # ---- pass 1a: load natural bf16 (cast during DMA), four quarter-batches ----
            QS = ST // 4
            nats = []
            for q in range(4):
                nat = nat_pool.tile([128, QS, H], BF16, tag="nat")
                if not (half_dma and q >= 2):
                    nc.gpsimd.dma_start(
                        nat[:, :, :], hid_t[b, :, q * QS:(q + 1) * QS, :]
                    )
                nats.append(nat)

            # ---- pass 1b: transpose to hiddenT bf16 [128h, ht, s] ----
            # loop sq outer so transposes consume quarter-batches as they land
            hT = ht_pool.tile([128, HT, S], BF16, tag="hT")
            if skip_transpose:
                nc.vector.memset(hT[:, 0, 0:16], 0.0)
            if not skip_transpose:
                for sq in range(4):          # groups of 4 s-tiles = one quarter
                    for ht in range(HT):
                        ptr = psum_tr.tile([128, 512], F32, tag="ptr")
                        for k in range(4):
                            st = sq * 4 + k
                            nc.tensor.matmul(
                                ptr[:, k * 128:(k + 1) * 128],
                                lhsT=nats[sq][:, k, ht * 128:(ht + 1) * 128],
                                rhs=ident_sb[:, :],
                                start=True, stop=True,
                            )
                        nc.vector.tensor_copy(
                            hT[:, ht, sq * 512:(sq + 1) * 512], ptr[:, :]
                        )

            # ---- pass 1c: mm1 + tanh + u accumulation ----
            e_sb = small_pool.tile([1, S], F32, tag="e")
            esum4 = small_pool.tile([1, SC], F32, tag="esum4")
            # mm1 + tanh for a whole s-chunk; u-mms for the PREVIOUS s-chunk
            # run as one dense burst of 8 accumulating matmuls, issued while
            # this chunk's mm1 stream keeps PE warm ahead of them.
            prev = None  # (pu_prev, sc_prev, [8 tanh tiles])

            def flush_prev():
                ppu, psc, ptanhs = prev
                for pg, ptanh in enumerate(ptanhs):
                    nc.tensor.matmul(
                        ppu[0:1, :], lhsT=wcol_sb[:, pg:pg + 1], rhs=ptanh[:, :],
                        start=(pg == 0), stop=(pg == GT - 1),
                    )
                nc.scalar.activation(
                    e_sb[0:1, psc * 512:(psc + 1) * 512], ppu[0:1, :], AF.Exp,
                    accum_out=esum4[0:1, psc:psc + 1],
                )

            for sc in range(SC):
                pu = psum_u.tile([1, 512], F32, tag="pu")
                tanhs = []
                for g in range(GT):
                    pth = psum_th.tile([128, 512], F32, tag="pth")
                    for h in range(HT):
                        nc.tensor.matmul(
                            pth[:, :],
                            lhsT=whT_sb[:, h, g * 128:(g + 1) * 128],
                            rhs=hT[:, h, sc * 512:(sc + 1) * 512],
                            start=(h == 0), stop=(h == HT - 1),
                        )
                    if g == 1 and prev is not None and not skip_umm:
                        flush_prev()
                    tanh_sb = tanh_pool.tile([128, 512], BF16, tag="tanh")
                    nc.scalar.activation(
                        tanh_sb[:, :], pth[:, :], AF.Tanh,
                        bias=whb_sb[:, g:g + 1],
                    )
                    tanhs.append(tanh_sb)
                prev = (pu, sc, tanhs)
            if not skip_umm:
                flush_prev()
            else:
                nc.vector.memset(e_sb[0:1, :], 1.0)
                nc.vector.memset(esum4[0:1, :], 1.0)
            prev = None

            # ---- softmax denominator ----
            if skip_pass2:
                r_dummy = small_pool.tile([1, H], F32, tag="r")
                nc.scalar.activation(r_dummy[0:1, :], e_sb[0:1, 0:H], AF.Copy)
                nc.sync.dma_start(out[b, 0:1, :], r_dummy[0:1, :])
                continue
            esum = small_pool.tile([1, 1], F32, tag="esum")
            nc.vector.tensor_reduce(
                esum[0:1, :], esum4[0:1, :], axis=mybir.AxisListType.X,
                op=mybir.AluOpType.add,
            )
            rz = small_pool.tile([1, 1], F32, tag="rz")
            nc.vector.reciprocal(rz[0:1, :], esum[0:1, :])

            # ---- reshape e [1, 2048] -> eT [128, 16] via DRAM bounce ----
            e_dram = dram_pool.tile([1, S], F32, tag="edram")
            nc.sync.dma_start(e_dram[0:1, :], e_sb[0:1, :])
            eT = small_pool.tile([128, ST], BF16, tag="eT")
            nc.gpsimd.dma_start(
                eT[:, :], e_dram[0:1, :].rearrange("a (t p) -> (a p) t", p=128)
            )

            # ---- pass 2: r_unnorm = e @ hidden from resident natural bf16 ----
            pr = psum_r.tile([1, H], F32, tag="pr")
            for st in range(ST):
                q, k = st // 4, st % 4
                for n in range(2):
                    nc.tensor.matmul(
                        pr[0:1, n * 512:(n + 1) * 512],
                        lhsT=eT[:, st:st + 1],
                        rhs=nats[q][:, k, n * 512:(n + 1) * 512],
                        start=(st == 0), stop=(st == ST - 1),
                    )
            r_sb = small_pool.tile([1, H], F32, tag="r")
            nc.scalar.activation(r_sb[0:1, :], pr[0:1, :], AF.Copy, scale=rz[0:1, :])
            nc.sync.dma_start(out[b, 0:1, :], r_sb[0:1, :])

    return nc


_NC_CACHE = None


def make_sharded_runner(nc):
    """Build a cached sharded-jit callable for `nc` (mirrors
    bass2jax.run_bass_via_pjrt) so repeated executions can be timed without
    re-jitting. Returns (fn, prep) where prep(in_maps) -> device args and
    fn(*args) -> out arrays."""
    import jax
    import numpy as _np
    from jax.sharding import Mesh, PartitionSpec
    from jax.experimental.shard_map import shard_map
    from concourse import bass2jax as b2j
    import concourse.mybir as _mybir

    b2j.install_neuronx_cc_hook()
    partition_name = nc.partition_id_tensor.name if nc.partition_id_tensor else None
    in_names, out_names, out_avals, zero_outs = [], [], [], []
    for alloc in nc.m.functions[0].allocations:
        if not isinstance(alloc, _mybir.MemoryLocationSet):
            continue
        name = alloc.memorylocations[0].name
        if alloc.kind == "ExternalInput":
            if name != partition_name:
                in_names.append(name)
        elif alloc.kind == "ExternalOutput":
            out_names.append(name)
            shape = tuple(alloc.tensor_shape)
            dtype = _mybir.dt.np(alloc.dtype)
            out_avals.append(jax.core.ShapedArray(shape, dtype))
            zero_outs.append(_np.zeros(shape, dtype))
    n_params = len(in_names)
    n_outs = len(out_avals)
    all_names = in_names + out_names
    if partition_name is not None:
        all_names.append(partition_name)
    donate = tuple(range(n_params, n_params + n_outs))

    def _body(*args):
        operands = list(args)
        if partition_name is not None:
            operands.append(b2j.partition_id_tensor())
        outs = b2j._bass_exec_p.bind(
            *operands,
            out_avals=tuple(out_avals),
            in_names=tuple(all_names),
            out_names=tuple(out_names),
            lowering_input_output_aliases=(),
            sim_require_finite=True,
            sim_require_nnan=True,
            nc=nc,
        )
        return tuple(outs)

    devices = jax.devices()[:NCORES]
    mesh = Mesh(np.asarray(devices), ("core",))
    in_specs = (PartitionSpec("core"),) * (n_params + n_outs)
    out_specs = (PartitionSpec("core"),) * n_outs
    fn = jax.jit(
        shard_map(_body, mesh=mesh, in_specs=in_specs, out_specs=out_specs,
                  check_rep=False),
        donate_argnums=donate, keep_unused=True,
    )

    def prep(in_maps):
        per_core = [[_np.asarray(m[name]) for name in in_names] for m in in_maps]
        concat_in = [
            _np.concatenate([per_core[c][i] for c in range(NCORES)], axis=0)
            for i in range(n_params)
        ]
        dev_in = [jax.device_put(x) for x in concat_in]
        return dev_in

    def zeros():
        return [np.zeros((NCORES * z.shape[0], *z.shape[1:]), z.dtype)
                for z in zero_outs]

    return fn, prep, zeros


def make_chained_runner(nc, k):
    """Like make_sharded_runner but executes the NEFF k times sequentially
    inside ONE jitted program — one tunnel dispatch, k on-device executions.
    Timing two k values isolates pure device time."""
    import jax
    import jax.numpy as jnp
    import numpy as _np
    from jax.sharding import Mesh, PartitionSpec
    from jax.experimental.shard_map import shard_map
    from concourse import bass2jax as b2j
    import concourse.mybir as _mybir

    b2j.install_neuronx_cc_hook()
    partition_name = nc.partition_id_tensor.name if nc.partition_id_tensor else None
    in_names, out_names, out_avals = [], [], []
    for alloc in nc.m.functions[0].allocations:
        if not isinstance(alloc, _mybir.MemoryLocationSet):
            continue
        name = alloc.memorylocations[0].name
        if alloc.kind == "ExternalInput":
            if name != partition_name:
                in_names.append(name)
        elif alloc.kind == "ExternalOutput":
            out_names.append(name)
            out_avals.append(jax.core.ShapedArray(
                tuple(alloc.tensor_shape), _mybir.dt.np(alloc.dtype)))
    n_params = len(in_names)
    all_names = in_names + out_names
    if partition_name is not None:
        all_names.append(partition_name)

    def _body(*args):
        ins = list(args[:n_params])
        outs = list(args[n_params:])
        for _ in range(k):
            operands = ins + outs          # prior outputs seed the out buffers
            if partition_name is not None:
                operands.append(b2j.partition_id_tensor())
            outs = list(b2j._bass_exec_p.bind(
                *operands,
                out_avals=tuple(out_avals),
                in_names=tuple(all_names),
                out_names=tuple(out_names),
                lowering_input_output_aliases=(),
                sim_require_finite=True,
                sim_require_nnan=True,
                nc=nc,
            ))
        return tuple(outs)

    devices = jax.devices()[:NCORES]
    mesh = Mesh(np.asarray(devices), ("core",))
    n_outs = len(out_names)
    in_specs = (PartitionSpec("core"),) * (n_params + n_outs)
    out_specs = (PartitionSpec("core"),) * n_outs
    fn = jax.jit(shard_map(_body, mesh=mesh, in_specs=in_specs,
                           out_specs=out_specs, check_rep=False))

    def prep(in_maps):
        per_core = [[_np.asarray(m[name]) for name in in_names] for m in in_maps]
        concat_in = [
            _np.concatenate([per_core[c][i] for c in range(NCORES)], axis=0)
            for i in range(n_params)
        ]
        concat_in += [
            _np.zeros((NCORES * av.shape[0], *av.shape[1:]), av.dtype)
            for av in out_avals
        ]
        return [jax.device_put(x) for x in concat_in]

    return fn, prep


def kernel(**inputs):
    global _NC_CACHE
    hidden = np.ascontiguousarray(np.asarray(inputs["hidden"], dtype=np.float32))
    Wh_w = np.asarray(inputs["Wh_w"], dtype=np.float32)
    Wh_b = np.asarray(inputs["Wh_b"], dtype=np.float32)
    w_w = np.asarray(inputs["w_w"], dtype=np.float32)

    whT_np = np.ascontiguousarray(Wh_w.T).astype(ml_dtypes.bfloat16)
    whb_np = np.ascontiguousarray(Wh_b.reshape(GT, 128))
    wcol_np = np.ascontiguousarray(w_w[0, :H].reshape(GT, 128)).astype(ml_dtypes.bfloat16)
    ident_np = np.eye(128, dtype=np.float32).astype(ml_dtypes.bfloat16)

    if _NC_CACHE is None:
        _NC_CACHE = build_kernel()
    nc = _NC_CACHE

    in_maps = []
    for k in range(NCORES):
        in_maps.append({
            "hidden": np.ascontiguousarray(hidden[k * BPC:(k + 1) * BPC]),
            "whT": whT_np,
            "whb": whb_np,
            "wcol": wcol_np,
            "ident": ident_np,
        })

    res = run_bass_kernel_spmd(nc, in_maps, core_ids=list(range(NCORES)))
    out = np.concatenate([r["out"] for r in res.results], axis=0)
    return out.astype(np.float32)


if __name__ == "__main__":
    rng = np.random.default_rng(0)
    test_inputs = {
        "hidden": rng.standard_normal((B, S, H), dtype=np.float32),
        "aspect": rng.standard_normal((B, 1, A), dtype=np.float32),
        "Wh_w": rng.standard_normal((H, H), dtype=np.float32) * 0.03,
        "Wh_b": rng.standard_normal((H,), dtype=np.float32) * 0.03,
        "Wv_w": rng.standard_normal((A, A), dtype=np.float32) * 0.06,
        "Wv_b": rng.standard_normal((A,), dtype=np.float32) * 0.06,
        "w_w": rng.standard_normal((1, H + A), dtype=np.float32) * 0.03,
        "w_b": rng.standard_normal((1,), dtype=np.float32) * 0.03,
    }
    r = kernel(**test_inputs)
    print("kernel out", r.shape, r.dtype, float(np.abs(r).max()))


# revision 40
# speedup vs baseline: 1.2924x; 1.0414x over previous
"""Trainium2 Bass kernel for nn_Attention_19877108646354 (aspect-attention pooling).

Math (per batch b):
    th = hidden[b] @ Wh_w.T + Wh_b            # [S, H]
    u  = tanh(th) @ w_w[0, :H]                # [S]   (aspect branch + w_b are
                                              #        constant per batch -> cancel in softmax)
    alpha = softmax(u)                        # [S]
    r[b]  = alpha @ hidden[b]                 # [H]

Sharding: data-parallel over batch, 4 batches per core on 8 cores.

On-device pipeline per batch:
  1. SWDGE cast-DMA: hidden[b] fp32 DRAM -> natural bf16 SBUF  [128s, 8x1024h] x2 halves
  2. PE: transpose via normal matmul against identity (stays HAM-warm):
       hiddenT[h-tile][128h, s] bf16, evicted PSUM->SBUF by DVE cast-copies
  3. PE mm1: th.T[g,s] = sum_h WhT[h,g-tile].T @ hiddenT -> PSUM [128g, 512s]
  4. ACT: tanh(th.T + Wh_b[g]) PSUM -> SBUF bf16
  5. PE u-mm: u[1, 512s] += w[g-tile].T @ tanh  (accumulate over g in PSUM)
  6. ACT: e = exp(u) (no max-shift needed, |u| <= ~1.5) with accum_out = sum(e)
  7. DVE: rz = 1/sum(e);  SWDGE strided DMA reshapes e [1,2048] -> eT [128,16]
  8. 2nd pass: HWDGE fp32 loads of hidden[b]; PE mm2 (float32r):
       r_unnorm[1, 1024] += eT[:, st].T @ hidden_tile
  9. ACT: r = r_unnorm * rz -> SBUF; DMA to output.
"""

from contextlib import ExitStack

import numpy as np
import ml_dtypes

import concourse.bass as bass
import concourse.tile as tile
import concourse.mybir as mybir
from concourse.bass_utils import run_bass_kernel_spmd

B, S, H, A = 32, 2048, 1024, 256
NCORES = 8
BPC = B // NCORES          # batches per core
ST = S // 128              # 16 s-tiles per batch
HT = H // 128              # 8 h-tiles
GT = H // 128              # 8 g-tiles
SC = S // 512              # 4 s-chunks of 512

F32 = mybir.dt.float32
F32R = mybir.dt.float32r
BF16 = mybir.dt.bfloat16
AF = mybir.ActivationFunctionType

_nop_uid = [0]


class SplitWaitTC(tile.TileContext):
    """TileContext variant for a walrus codegen that accepts at most ONE sync
    wait per instruction: extra waits are peeled onto same-engine NoOps placed
    immediately before the instruction (semantically identical), and the tail
    drain's many-lane wait set is spread over SP NoOps."""

    def _add_instruction(self, inst):
        si = inst.sync_info
        if si is not None and len(si.on_wait) > 1:
            waits = list(si.on_wait)
            for w in waits[:-1]:
                _nop_uid[0] += 1
                nop = mybir.InstNoOp(
                    name=f"waitsplit_{_nop_uid[0]}",
                    sync_info=mybir.SyncInfo(on_wait=[w], on_update=[]),
                    bass_nofuse=True,
                    engine=inst.engine,
                )
                super()._add_instruction(nop)
            inst.sync_info = mybir.SyncInfo(
                on_wait=[waits[-1]], on_update=list(si.on_update)
            )
        super()._add_instruction(inst)

    def _drain_and_barrier(self, tick_clock, wait_clock):
        from concourse.vector_clock import ScopedClock

        drain_inst = self.nc.sync.drain()
        wait_clock.add_sem_waits(
            drain_inst.ins, ScopedClock({None: tick_clock.global_clock})
        )
        si = drain_inst.ins.sync_info
        if si is not None and len(si.on_wait) > 1:
            waits = list(si.on_wait)
            drain_inst.ins.sync_info = mybir.SyncInfo(
                on_wait=[waits[0]], on_update=list(si.on_update)
            )
            for w in waits[1:]:
                nop = self.nc.sync.nop(nofuse=True, hint="drain_split")
                nop.ins.sync_info = mybir.SyncInfo(on_wait=[w], on_update=[])

        self.nc.all_engine_barrier()
        assert self.sems is not None
        popped = self.nc._tile_sem_poison_stack.pop()
        assert popped is self._sem_poison
        self.nc.clear_and_free_semaphores(list(self.sems.allocated().values()))
        self.nc.all_engine_barrier()


def build_kernel(reps=1, skip_transpose=False, skip_pass2=False, skip_umm=False,
                 half_dma=False):
    ablation = skip_transpose or skip_pass2 or skip_umm or half_dma
    nc = bass.Bass(trn_type="TRN2")

    hid = nc.dram_tensor("hidden", [BPC, S, H], F32, kind="ExternalInput")
    whT = nc.dram_tensor("whT", [H, H], BF16, kind="ExternalInput")       # WhT[h, g] = Wh_w[g, h]
    whb = nc.dram_tensor("whb", [GT, 128], F32, kind="ExternalInput")     # whb[gt, p] = Wh_b[gt*128+p]
    wcol = nc.dram_tensor("wcol", [GT, 128], BF16, kind="ExternalInput")  # wcol[gt, p] = w_w[0, gt*128+p]
    ident = nc.dram_tensor("ident", [128, 128], BF16, kind="ExternalInput")
    ones = nc.dram_tensor("ones", [128, 1], F32, kind="ExternalInput")
    out = nc.dram_tensor("out", [BPC, 1, H], F32, kind="ExternalOutput")

    with SplitWaitTC(nc) as tc, ExitStack() as ctx:
        if ablation:
            tc.race_detector_enabled = False
        consts = ctx.enter_context(tc.tile_pool(name="consts", bufs=1))
        nat_pool = ctx.enter_context(tc.tile_pool(name="nat", bufs=9))
        ht_pool = ctx.enter_context(tc.tile_pool(name="hT", bufs=1))
        tanh_pool = ctx.enter_context(tc.tile_pool(name="tanh", bufs=18))
        small_pool = ctx.enter_context(tc.tile_pool(name="small", bufs=2))
        psum_tr = ctx.enter_context(tc.tile_pool(name="ptr", bufs=2, space="PSUM"))
        psum_th = ctx.enter_context(tc.tile_pool(name="pth", bufs=2, space="PSUM"))
        psum_ut = ctx.enter_context(tc.tile_pool(name="puT", bufs=2, space="PSUM"))
        psum_r = ctx.enter_context(tc.tile_pool(name="pr", bufs=1, space="PSUM"))

        # --- load constants ---
        whT_sb = consts.tile([128, HT, H], BF16)      # [p(h), ht, g]
        for ht in range(HT):
            nc.sync.dma_start(whT_sb[:, ht, :], whT[ht * 128:(ht + 1) * 128, :])
        whb_sb = consts.tile([128, GT], F32)          # [p(g), gt]
        nc.sync.dma_start(whb_sb[:, :], whb.rearrange("g p -> p g"))
        wcol_sb = consts.tile([128, GT], BF16)
        nc.sync.dma_start(wcol_sb[:, :], wcol.rearrange("g p -> p g"))
        ident_sb = consts.tile([128, 128], BF16)
        nc.sync.dma_start(ident_sb[:, :], ident[:, :])
        ones_sb = consts.tile([128, 1], F32)
        nc.sync.dma_start(ones_sb[:, :], ones[:, :])

        # hid[b] viewed as [p(s within tile), s-tile, h]
        hid_t = hid.rearrange("b (u p) h -> b p u h", p=128)

        tail = None
        for b_iter in range(BPC * reps):
            b = b_iter % BPC
            # ---- pass 1a: load natural bf16 (cast during DMA), four quarter-batches ----
            QS = ST // 4
            nats = []
            for q in range(4):
                nat = nat_pool.tile([128, QS, H], BF16, tag="nat")
                if not (half_dma and q >= 2):
                    nc.gpsimd.dma_start(
                        nat[:, :, :], hid_t[b, :, q * QS:(q + 1) * QS, :]
                    )
                nats.append(nat)

            # ---- pass 1b: transpose to hiddenT bf16 [128h, ht, s] ----
            # loop sq outer so transposes consume quarter-batches as they land
            hT = ht_pool.tile([128, HT, S], BF16, tag="hT")
            if skip_transpose:
                nc.vector.memset(hT[:, 0, 0:16], 0.0)
            if not skip_transpose:
                for sq in range(4):          # groups of 4 s-tiles = one quarter
                    for ht in range(HT):
                        ptr = psum_tr.tile([128, 512], F32, tag="ptr")
                        for k in range(4):
                            st = sq * 4 + k
                            nc.tensor.matmul(
                                ptr[:, k * 128:(k + 1) * 128],
                                lhsT=nats[sq][:, k, ht * 128:(ht + 1) * 128],
                                rhs=ident_sb[:, :],
                                start=True, stop=True,
                            )
                        nc.vector.tensor_copy(
                            hT[:, ht, sq * 512:(sq + 1) * 512], ptr[:, :]
                        )

            # ---- pass 1c: mm1 + tanh; u accumulated TRANSPOSED: uT[128s, st] ----
            # u-burst for s-chunk sc: 32 matmuls (M=128, N=1) contracting the
            # g-partition of stored tanh tiles against w columns, accumulating
            # into puT columns. Lands u directly in s-partition layout, so exp
            # emits eT [128, 16] with no DRAM bounce.
            puT = psum_ut.tile([128, ST], F32, tag="puT")

            def flush_uT(pput, psc, ptanhs):
                for k in range(4):
                    col = psc * 4 + k
                    for g in range(GT):
                        nc.tensor.matmul(
                            pput[:, col:col + 1],
                            lhsT=ptanhs[g][:, k * 128:(k + 1) * 128],
                            rhs=wcol_sb[:, g:g + 1],
                            start=(g == 0), stop=(g == GT - 1),
                        )

            prev_sc = None  # (sc, [8 tanh tiles])
            for sc in range(SC):
                tanhs = []
                for g in range(GT):
                    pth = psum_th.tile([128, 512], F32, tag="pth")
                    for h in range(HT):
                        nc.tensor.matmul(
                            pth[:, :],
                            lhsT=whT_sb[:, h, g * 128:(g + 1) * 128],
                            rhs=hT[:, h, sc * 512:(sc + 1) * 512],
                            start=(h == 0), stop=(h == HT - 1),
                        )
                    if sc == 0 and g == 1 and tail is not None:
                        tail()          # previous batch: mm2 + scale + out
                    if g == 1 and prev_sc is not None and not skip_umm:
                        flush_uT(puT, *prev_sc)
                    tanh_sb = tanh_pool.tile([128, 512], BF16, tag="tanh")
                    nc.scalar.activation(
                        tanh_sb[:, :], pth[:, :], AF.Tanh,
                        bias=whb_sb[:, g:g + 1],
                    )
                    tanhs.append(tanh_sb)
                prev_sc = (sc, tanhs)

            def make_tail(b, puT, prev_sc, nats):
                def tail():
                    eT = small_pool.tile([128, ST], BF16, tag="eT")
                    acc = small_pool.tile([128, 1], F32, tag="acc")
                    rz = small_pool.tile([1, 1], F32, tag="rz")
                    pr = psum_r.tile([1, H], F32, tag="pr")
                    if not skip_umm:
                        flush_uT(puT, *prev_sc)
                        nc.scalar.activation(
                            eT[:, :], puT[:, :], AF.Exp, accum_out=acc[:, :]
                        )
                        # esum = ones . acc via a tiny matmul into pr[0,0]
                        nc.tensor.matmul(
                            pr[0:1, 0:1], lhsT=ones_sb[:, :], rhs=acc[:, :],
                            start=True, stop=True,
                        )
                        nc.vector.reciprocal(rz[0:1, :], pr[0:1, 0:1])
                    else:
                        nc.vector.memset(eT[:, :], 1.0)
                        nc.vector.memset(rz[0:1, :], 1.0)
                    r_sb = small_pool.tile([1, H], F32, tag="r")
                    if skip_pass2:
                        nc.vector.memset(r_sb[0:1, :], 0.0)
                        nc.sync.dma_start(out[b, 0:1, :], r_sb[0:1, :])
                        return
                    for st in range(ST):
                        q, k = st // 4, st % 4
                        for n in range(2):
                            nc.tensor.matmul(
                                pr[0:1, n * 512:(n + 1) * 512],
                                lhsT=eT[:, st:st + 1],
                                rhs=nats[q][:, k, n * 512:(n + 1) * 512],
                                start=(st == 0), stop=(st == ST - 1),
                            )
                    nc.scalar.activation(
                        r_sb[0:1, :], pr[0:1, :], AF.Copy, scale=rz[0:1, :]
                    )
                    nc.sync.dma_start(out[b, 0:1, :], r_sb[0:1, :])
                return tail

            tail = make_tail(b, puT, prev_sc, nats)
        tail()
        tail = None

    return nc


_NC_CACHE = None


def make_sharded_runner(nc):
    """Build a cached sharded-jit callable for `nc` (mirrors
    bass2jax.run_bass_via_pjrt) so repeated executions can be timed without
    re-jitting. Returns (fn, prep) where prep(in_maps) -> device args and
    fn(*args) -> out arrays."""
    import jax
    import numpy as _np
    from jax.sharding import Mesh, PartitionSpec
    from jax.experimental.shard_map import shard_map
    from concourse import bass2jax as b2j
    import concourse.mybir as _mybir

    b2j.install_neuronx_cc_hook()
    partition_name = nc.partition_id_tensor.name if nc.partition_id_tensor else None
    in_names, out_names, out_avals, zero_outs = [], [], [], []
    for alloc in nc.m.functions[0].allocations:
        if not isinstance(alloc, _mybir.MemoryLocationSet):
            continue
        name = alloc.memorylocations[0].name
        if alloc.kind == "ExternalInput":
            if name != partition_name:
                in_names.append(name)
        elif alloc.kind == "ExternalOutput":
            out_names.append(name)
            shape = tuple(alloc.tensor_shape)
            dtype = _mybir.dt.np(alloc.dtype)
            out_avals.append(jax.core.ShapedArray(shape, dtype))
            zero_outs.append(_np.zeros(shape, dtype))
    n_params = len(in_names)
    n_outs = len(out_avals)
    all_names = in_names + out_names
    if partition_name is not None:
        all_names.append(partition_name)
    donate = tuple(range(n_params, n_params + n_outs))

    def _body(*args):
        operands = list(args)
        if partition_name is not None:
            operands.append(b2j.partition_id_tensor())
        outs = b2j._bass_exec_p.bind(
            *operands,
            out_avals=tuple(out_avals),
            in_names=tuple(all_names),
            out_names=tuple(out_names),
            lowering_input_output_aliases=(),
            sim_require_finite=True,
            sim_require_nnan=True,
            nc=nc,
        )
        return tuple(outs)

    devices = jax.devices()[:NCORES]
    mesh = Mesh(np.asarray(devices), ("core",))
    in_specs = (PartitionSpec("core"),) * (n_params + n_outs)
    out_specs = (PartitionSpec("core"),) * n_outs
    fn = jax.jit(
        shard_map(_body, mesh=mesh, in_specs=in_specs, out_specs=out_specs,
                  check_rep=False),
        donate_argnums=donate, keep_unused=True,
    )

    def prep(in_maps):
        per_core = [[_np.asarray(m[name]) for name in in_names] for m in in_maps]
        concat_in = [
            _np.concatenate([per_core[c][i] for c in range(NCORES)], axis=0)
            for i in range(n_params)
        ]
        dev_in = [jax.device_put(x) for x in concat_in]
        return dev_in

    def zeros():
        return [np.zeros((NCORES * z.shape[0], *z.shape[1:]), z.dtype)
                for z in zero_outs]

    return fn, prep, zeros


def make_chained_runner(nc, k):
    """Like make_sharded_runner but executes the NEFF k times sequentially
    inside ONE jitted program — one tunnel dispatch, k on-device executions.
    Timing two k values isolates pure device time."""
    import jax
    import jax.numpy as jnp
    import numpy as _np
    from jax.sharding import Mesh, PartitionSpec
    from jax.experimental.shard_map import shard_map
    from concourse import bass2jax as b2j
    import concourse.mybir as _mybir

    b2j.install_neuronx_cc_hook()
    partition_name = nc.partition_id_tensor.name if nc.partition_id_tensor else None
    in_names, out_names, out_avals = [], [], []
    for alloc in nc.m.functions[0].allocations:
        if not isinstance(alloc, _mybir.MemoryLocationSet):
            continue
        name = alloc.memorylocations[0].name
        if alloc.kind == "ExternalInput":
            if name != partition_name:
                in_names.append(name)
        elif alloc.kind == "ExternalOutput":
            out_names.append(name)
            out_avals.append(jax.core.ShapedArray(
                tuple(alloc.tensor_shape), _mybir.dt.np(alloc.dtype)))
    n_params = len(in_names)
    all_names = in_names + out_names
    if partition_name is not None:
        all_names.append(partition_name)

    def _body(*args):
        ins = list(args[:n_params])
        outs = list(args[n_params:])
        for _ in range(k):
            operands = ins + outs          # prior outputs seed the out buffers
            if partition_name is not None:
                operands.append(b2j.partition_id_tensor())
            outs = list(b2j._bass_exec_p.bind(
                *operands,
                out_avals=tuple(out_avals),
                in_names=tuple(all_names),
                out_names=tuple(out_names),
                lowering_input_output_aliases=(),
                sim_require_finite=True,
                sim_require_nnan=True,
                nc=nc,
            ))
        return tuple(outs)

    devices = jax.devices()[:NCORES]
    mesh = Mesh(np.asarray(devices), ("core",))
    n_outs = len(out_names)
    in_specs = (PartitionSpec("core"),) * (n_params + n_outs)
    out_specs = (PartitionSpec("core"),) * n_outs
    fn = jax.jit(shard_map(_body, mesh=mesh, in_specs=in_specs,
                           out_specs=out_specs, check_rep=False))

    def prep(in_maps):
        per_core = [[_np.asarray(m[name]) for name in in_names] for m in in_maps]
        concat_in = [
            _np.concatenate([per_core[c][i] for c in range(NCORES)], axis=0)
            for i in range(n_params)
        ]
        concat_in += [
            _np.zeros((NCORES * av.shape[0], *av.shape[1:]), av.dtype)
            for av in out_avals
        ]
        return [jax.device_put(x) for x in concat_in]

    return fn, prep


def kernel(**inputs):
    global _NC_CACHE
    hidden = np.ascontiguousarray(np.asarray(inputs["hidden"], dtype=np.float32))
    Wh_w = np.asarray(inputs["Wh_w"], dtype=np.float32)
    Wh_b = np.asarray(inputs["Wh_b"], dtype=np.float32)
    w_w = np.asarray(inputs["w_w"], dtype=np.float32)

    whT_np = np.ascontiguousarray(Wh_w.T).astype(ml_dtypes.bfloat16)
    whb_np = np.ascontiguousarray(Wh_b.reshape(GT, 128))
    wcol_np = np.ascontiguousarray(w_w[0, :H].reshape(GT, 128)).astype(ml_dtypes.bfloat16)
    ident_np = np.eye(128, dtype=np.float32).astype(ml_dtypes.bfloat16)
    ones_np = np.ones((128, 1), dtype=np.float32)

    if _NC_CACHE is None:
        _NC_CACHE = build_kernel()
    nc = _NC_CACHE

    in_maps = []
    for k in range(NCORES):
        in_maps.append({
            "hidden": np.ascontiguousarray(hidden[k * BPC:(k + 1) * BPC]),
            "whT": whT_np,
            "whb": whb_np,
            "wcol": wcol_np,
            "ident": ident_np,
            "ones": ones_np,
        })

    res = run_bass_kernel_spmd(nc, in_maps, core_ids=list(range(NCORES)))
    out = np.concatenate([r["out"] for r in res.results], axis=0)
    return out.astype(np.float32)


if __name__ == "__main__":
    rng = np.random.default_rng(0)
    test_inputs = {
        "hidden": rng.standard_normal((B, S, H), dtype=np.float32),
        "aspect": rng.standard_normal((B, 1, A), dtype=np.float32),
        "Wh_w": rng.standard_normal((H, H), dtype=np.float32) * 0.03,
        "Wh_b": rng.standard_normal((H,), dtype=np.float32) * 0.03,
        "Wv_w": rng.standard_normal((A, A), dtype=np.float32) * 0.06,
        "Wv_b": rng.standard_normal((A,), dtype=np.float32) * 0.06,
        "w_w": rng.standard_normal((1, H + A), dtype=np.float32) * 0.03,
        "w_b": rng.standard_normal((1,), dtype=np.float32) * 0.03,
    }
    r = kernel(**test_inputs)
    print("kernel out", r.shape, r.dtype, float(np.abs(r).max()))
